# revision 1
# baseline (speedup 1.0000x reference)
"""Trainium2 Bass kernel for ContrastiveMSELoss.

Reference computes, over all N^2 pairs (diagonal masked to 0):
    mse_ij  = (|x_i|^2 + |x_j|^2 - 2 x_i.x_j) / D
    sign_ij = +1 if class_i == class_j else -1
    loss    = mean_ij(sign_ij * mse_ij) + BETA

Using sum_{i,j in c} x_i.x_j = |M_c|^2 with M_c = sum_{i in c} x_i, the
loss collapses to class-bucketed first/second moments (O(N*D) work,
memory-bound -- no N x N gram matrix needed):

    T_same = sum_c (2 n_c SQ_c - 2 |M_c|^2) / D      (diag terms are 0)
    T_all  = (2 N SQ - 2 |M|^2) / D
    loss   = (2 T_same - T_all) / N^2 + BETA

Sharding: rows are split across 8 cores (1024 rows each).  Per core the
shard maps row r = p*8 + k onto partition p, sub-chunk k, so every DMA
block is contiguous per partition (2 KB descriptors, per-SDMA-engine
line rate).  The host pre-builds the one-hot class matrix (from the
tiny classes input) and ships it as a bf16 tensor; x blocks are spread
over the two HWDGE rings.  A one-hot matmul accumulates per-class sums
of X and X^2 in PSUM (separate 256-wide matmuls so the X side never
waits on the squares), with even sub-chunks targeting PSUM partitions
0:40 and odd ones 64:104 so consecutive matmuls use different PE
column groups and run concurrently.  One DVE copy folds PSUM 0:104 to
SBUF, and the result store runs after the TileContext with nothing
waiting on its HBM write receipt -- the NEFF epilogue's DMA drain
covers it.  Host combines the per-core [104, 512] partials (rows 40:64
are padding).
"""

import numpy as np

import concourse.bacc as bacc
import concourse.bass as bass
import concourse.tile as tile
from concourse import mybir
from concourse.bass_utils import run_bass_kernel_spmd

N, D = 8192, 256
N_CORES = 8
ROWS = N // N_CORES          # 1024 rows per core
P = 128                      # partitions
K = ROWS // P                # 8 sub-rows per partition (row = p*K + k)
NCLS = 40
BETA = 1.0
OUTP = 104                   # output partitions: rows 0:40 + padding + 64:104

_CACHE = {}


def _build_bass():
    nc = bacc.Bacc(
        "TRN2",
        target_bir_lowering=False,
        debug=False,
        enable_asserts=False,
        num_devices=N_CORES,
    )
    # x shard viewed as [128, 8, 256]: partition p = rows p*8 .. p*8+7
    x = nc.dram_tensor("x", [P, K, D], mybir.dt.float32, kind="ExternalInput")
    # host-built one-hot: ohd[p, k, c] = (class[p*8+k] == c)
    ohd = nc.dram_tensor(
        "oh", [P, K, NCLS], mybir.dt.bfloat16, kind="ExternalInput"
    )
    # stats rows 0:40 = even-k chain, 64:104 = odd-k chain, 40:64 garbage;
    # cols 0:256 per-class sums of x, 256:512 per-class sums of x^2
    stats = nc.dram_tensor(
        "stats", [OUTP, 2 * D], mybir.dt.bfloat16, kind="ExternalOutput"
    )

    sem_out = nc.alloc_semaphore("out_dma")
    sem_fold = nc.alloc_semaphore("fold_done")
    # raw (non-Tile) tensors so the post-TileContext store has concrete
    # access patterns.  X and X^2 accumulate in separate PSUM banks so the
    # X-side fold can overlap the remaining X^2 matmuls (no same-bank
    # PE-write / DVE-read hazard across banks).
    out_sb = nc.alloc_sbuf_tensor("out_sb_raw", [P, 2 * D], mybir.dt.bfloat16)
    accx = nc.alloc_psum_tensor("accx_raw", [P, D], mybir.dt.float32)
    accs = nc.alloc_psum_tensor("accs_raw", [P, D], mybir.dt.float32)

    with tile.TileContext(nc) as tc:
        with (
            tc.tile_pool(name="work", bufs=1) as work,
            tc.tile_pool(name="psum", bufs=1, space="PSUM") as psum_pool,
        ):
            xbx = work.tile([P, K, D], mybir.dt.bfloat16, tag="xbx")
            xbs = work.tile([P, K, D], mybir.dt.bfloat16, tag="xbs")
            oh = work.tile([P, K, NCLS], mybir.dt.bfloat16, tag="oh")

            # Input DMAs.  x rides the gpsimd SWDGE ring as two 512 KB
            # halves, casting f32 -> bf16 in flight.  SWDGE descriptor
            # emission is the stream's pacer at ~105 packets/us and the
            # descriptor count is per-DMA (128, one per partition), not
            # per-byte -- so two 4KB-per-partition DMAs emit half the
            # descriptors of four 2KB ones.  The one-hot rides the sync
            # HWDGE ring.
            nc.sync.dma_start(out=oh[:, :, :], in_=ohd[:, :, :])
            blocks = [(0, 4), (4, 4)]
            for k0, nk in blocks:
                nc.gpsimd.dma_start(
                    out=xbx[:, k0 : k0 + nk, :], in_=x[:, k0 : k0 + nk, :]
                )

            for k0, nk in blocks:
                # X matmuls first: they only need the streamed bf16 data
                for k in range(k0, k0 + nk):
                    lo = 0 if k % 2 == 0 else 64
                    nc.tensor.matmul(
                        accx[lo : lo + NCLS, :],
                        oh[:, k, :],
                        xbx[:, k, :],
                        start=(k < 2),
                        stop=(k >= K - 2),
                        skip_group_check=True,
                    )
                # squares on DVE (bf16 2x mode)
                nc.vector.tensor_mul(
                    xbs[:, k0 : k0 + nk, :],
                    xbx[:, k0 : k0 + nk, :],
                    xbx[:, k0 : k0 + nk, :],
                )
                for k in range(k0, k0 + nk):
                    lo = 0 if k % 2 == 0 else 64
                    nc.tensor.matmul(
                        accs[lo : lo + NCLS, :],
                        oh[:, k, :],
                        xbs[:, k, :],
                        start=(k < 2),
                        stop=(k >= K - 2),
                        skip_group_check=True,
                    )

            # split folds: the X-side fold runs as soon as the last X
            # matmul retires, overlapping the X^2 matmuls still running in
            # the other PSUM bank; partitions 0:104 in one op each
            # (partition count does not change DVE time; 40:64 is garbage)
            nc.vector.tensor_copy(out_sb[:OUTP, :D], accx[:OUTP, :]).then_inc(
                sem_fold, 1
            )
            nc.vector.tensor_copy(out_sb[:OUTP, D:], accs[:OUTP, :]).then_inc(
                sem_fold, 1
            )

    # Deliberately nothing waits on sem_out: the NEFF epilogue drains the
    # DMA rings before execution completes, which guarantees the store has
    # landed by the time the host reads `stats` (run_device also retries
    # once if a previous session left the device needing a reset).
    nc.sync.wait_ge(sem_fold, 2)
    nc.sync.dma_start(out=stats[:, :], in_=out_sb[:OUTP, :]).then_inc(sem_out, 16)

    return nc


def _get_nc():
    if "nc" not in _CACHE:
        nc = _build_bass()
        nc.finalize()
        _CACHE["nc"] = nc
    return _CACHE["nc"]


def run_device(output, classes, **spmd_kwargs):
    """Run the per-core Bass kernel; returns (list of per-core stats, results)."""
    x = np.ascontiguousarray(np.asarray(output), dtype=np.float32)
    cls = np.asarray(classes).astype(np.int64)
    onehot = (cls[:, None] == np.arange(NCLS)[None, :]).astype(np.float32)
    from ml_dtypes import bfloat16

    onehot = onehot.astype(bfloat16)
    in_maps = []
    for s in range(N_CORES):
        xs = x[s * ROWS : (s + 1) * ROWS].reshape(P, K, D)
        ohs = onehot[s * ROWS : (s + 1) * ROWS].reshape(P, K, NCLS)
        in_maps.append(
            {"x": np.ascontiguousarray(xs), "oh": np.ascontiguousarray(ohs)}
        )
    try:
        res = run_bass_kernel_spmd(
            _get_nc(), in_maps, core_ids=list(range(N_CORES)), **spmd_kwargs
        )
    except Exception:
        # a previous session can leave the device needing one reset cycle;
        # a single retry recovers it
        res = run_bass_kernel_spmd(
            _get_nc(), in_maps, core_ids=list(range(N_CORES)), **spmd_kwargs
        )
    stats = [res.results[s]["stats"] for s in range(N_CORES)]
    return stats, res


def _combine(stats, classes):
    """Combine per-core partial class stats into the scalar loss (float64)."""
    tot = np.sum(np.asarray(stats, dtype=np.float64), axis=0)  # [104, 512]
    tot = tot[:NCLS] + tot[64 : 64 + NCLS]                     # [40, 512]
    M_c = tot[:, :D]                                           # class sums
    SQ_c = tot[:, D:].sum(axis=1)                              # class |x|^2 sums
    n_c = np.bincount(np.asarray(classes).astype(np.int64), minlength=NCLS).astype(
        np.float64
    )
    SQ = SQ_c.sum()
    M = M_c.sum(axis=0)
    T_same = (2.0 * (n_c * SQ_c).sum() - 2.0 * (M_c * M_c).sum()) / D
    T_all = (2.0 * N * SQ - 2.0 * (M @ M)) / D
    loss = (2.0 * T_same - T_all) / (float(N) * float(N)) + BETA
    return np.float32(loss)


def kernel(output, classes):
    stats, _ = run_device(output, classes)
    return _combine(stats, classes)

